# revision 32
# baseline (speedup 1.0000x reference)
"""Trainium2 Bass kernel for GQA attention (B=1, S=2048, D=2048, H=16, KVH=4, HD=128).

Strategy, driven by this environment's measured cost model (per *unique*
instruction ~40-90us, per loop-executed instruction ~5-15us, DMA ~50us,
collective ~1.2ms floor; actual FLOPs/bytes nearly free):

  - Tensor parallel over heads (core c: q-heads {2c, 2c+1}, kv-head c//2),
    which minimizes per-core matmul work; one bf16 ReduceScatter of the
    transposed output at the end (host reassembles + transposes).
  - Nearly the whole kernel lives inside nested hardware loops (tc.For_i)
    with STATIC SBUF/PSUM addresses: moving operands use register offsets
    (bass.ds), and matmul stationary operands - which walrus cannot
    register-offset - are staged into fixed tiles with DVE copies whose
    *source* is register-offset. This shrinks the instruction stream from
    ~2000 to ~150 instructions.
  - PSUM accumulation across loop iterations uses memset + start=False.
  - bf16 everywhere on the matmul path (fp32 PSUM), host pre-casts/packs
    so each input is a single contiguous DMA.
  - RoPE head-dim permutation trick: wq/wk columns permuted per head to
    [even|odd] so RoPE is two contiguous 64-partition halves.
  - Causality via a full additive mask tensor indexed by (ktile, qchunk)
    inside the loops (identical on all cores).
"""

import numpy as np
import ml_dtypes
from contextlib import ExitStack

import concourse.bacc as bacc
import concourse.bass as bass
import concourse.tile as tile
import concourse.mybir as mybir
from concourse.bass_utils import run_bass_kernel_spmd

S = 2048
D = 2048
H = 16
KVH = 4
HD = 128
NCORES = 8
F32 = mybir.dt.float32
BF16 = mybir.dt.bfloat16
NPBF16 = ml_dtypes.bfloat16
SCALE = float(1.0 / np.sqrt(HD))
NEG = -1e9

_BUILD_CACHE = {}


def _emit_body(nc, tc, io):
    mm = nc.tensor.matmul
    ds = bass.ds
    with ExitStack() as ctx:
        sb = ctx.enter_context(tc.tile_pool(name="sb", bufs=1))
        dram = ctx.enter_context(tc.tile_pool(name="dram", bufs=1, space="DRAM"))

        ones_sb = sb.tile([128, 1], BF16, tag="ones")
        nc.vector.memset(ones_sb[:], 1.0)
        ident_sb = sb.tile([128, 128], BF16, tag="ident")
        nc.sync.dma_start(out=ident_sb[:], in_=io["ident"][:])
        wqkv_sb = sb.tile([128, 8192], BF16, tag="wqkv")
        nc.sync.dma_start(out=wqkv_sb[:], in_=io["wqkv"][:])
        wo_sb = sb.tile([128, 4096], BF16, tag="wo")
        nc.sync.dma_start(out=wo_sb[:], in_=io["wo2"][:])
        cossin_sb = sb.tile([64, 4096], BF16, tag="cossin")
        nc.sync.dma_start(out=cossin_sb[:], in_=io["cossin"][:])
        # mask_ext variant v = qc-grp+3: v=3 boundary-quad mask, v>3 zeros
        mask_sb = sb.tile([128, 16384], BF16, tag="mask")
        nc.sync.dma_start(out=mask_sb[:], in_=io["maskF"][:])
        xT_sb = sb.tile([128, 32768], BF16, tag="xT")
        nc.sync.dma_start(
            out=xT_sb[:].rearrange("p (t s) -> p t s", t=16),
            in_=io["xT"].rearrange("(t p) s -> p t s", p=128))

        # persistent activations: qk = [q0 | q1 | kT] each [128, 2048]
        qk_sb = sb.tile([128, 3, 2048], BF16, tag="qk")
        vT_sb = sb.tile([128, 2048], BF16, tag="vT")
        v_sb = sb.tile([128, 2048], BF16, tag="v")
        attnT_sb = sb.tile([128, 4096], BF16, tag="attnT")   # [hd, (h, q)]
        recip_sb = sb.tile([1, 4096], F32, tag="recip")

        xT_r = io["xT"].rearrange("(t p) s -> p t s", p=128)  # [128, 16, 2048]

        # ---------------- phase 1: QKV projections (transposed) --------------
        # loops (sc x dt): stage wqkv[dt], 4 MMs vs resident xT
        with tc.tile_pool(name="p1s", bufs=1) as p1s, \
             tc.tile_pool(name="pp1", bufs=1, space="PSUM") as pp1:
            ps_p = pp1.tile([128, 4, 512], F32, tag="ps_p")  # q0,q1,k,v
            wst = p1s.tile([128, 512], BF16, tag="wst")

            def p1_dt(sc, dt, unroll=1):
                nc.vector.tensor_copy(wst[:], wqkv_sb[:, ds(dt * 512, 512)])
                for j in range(4):
                    mm(ps_p[:, j, :], wst[:, j * 128:(j + 1) * 128],
                       xT_sb[:, ds(dt * 2048 + sc * 512, 512)],
                       start=False, stop=False)

            def p1_body(sc, unroll=1):
                nc.vector.memset(ps_p[:], 0.0)
                with tc.For_i(0, 16, 1) as dt:
                    p1_dt(sc, dt)
                for j in range(3):
                    nc.vector.tensor_copy(qk_sb[:, j, ds(sc * 512, 512)],
                                          ps_p[:, j, :])
                nc.vector.tensor_copy(vT_sb[:, ds(sc * 512, 512)], ps_p[:, 3, :])

            with tc.For_i(0, 4, 1) as sc:
                p1_body(sc)

        # ---------------- phase 1b: vT -> v natural (PE transposes) ----------
        with tc.tile_pool(name="p2s", bufs=1) as p2s, \
             tc.tile_pool(name="pp2", bufs=1, space="PSUM") as pp2:
            tst = p2s.tile([128, 128], BF16, tag="tst")
            ps_t = pp2.tile([128, 128], BF16, tag="ps_t")

            def p2_body(kt, unroll=1):
                nc.vector.tensor_copy(tst[:], vT_sb[:, ds(kt * 128, 128)])
                nc.tensor.transpose(ps_t[:], tst[:], ident_sb[:])
                nc.vector.tensor_copy(v_sb[:, ds(kt * 128, 128)], ps_t[:])

            with tc.For_i(0, 16, 1) as kt:
                p2_body(kt)

        # ---------------- phase 1.5: RoPE on q0, q1, k (in place) ------------
        # rolled over j in {q0, q1, k}: halves swap trick on [64, 2048] slabs
        qk_flat = qk_sb[:].rearrange("p a b -> p (a b)")
        qk_lo = qk_sb[0:64, :, :].rearrange("p a b -> p (a b)")
        qk_hi = qk_sb[64:128, :, :].rearrange("p a b -> p (a b)")
        with tc.tile_pool(name="rp", bufs=1) as rp:
            cosv = cossin_sb[:, 0:2048]
            sinv = cossin_sb[:, 2048:4096]
            t1c = rp.tile([64, 2048], BF16, tag="t1c")
            o1 = rp.tile([64, 2048], BF16, tag="o1")
            tmp = rp.tile([64, 2048], BF16, tag="tmp")

            def rope_body(j):
                t0 = qk_lo[:, j * 2048:(j + 1) * 2048]
                nc.sync.dma_start(out=t1c[:], in_=qk_hi[:, j * 2048:(j + 1) * 2048])
                nc.vector.tensor_mul(o1[:], t0, sinv)
                nc.vector.tensor_mul(tmp[:], t1c[:], cosv)
                nc.vector.tensor_add(o1[:], o1[:], tmp[:])
                nc.vector.tensor_mul(tmp[:], t1c[:], sinv)
                nc.vector.tensor_mul(t1c[:], t0, cosv)
                nc.vector.tensor_sub(qk_lo[:, j * 2048:(j + 1) * 2048],
                                     t1c[:], tmp[:])
                nc.sync.dma_start(out=qk_hi[:, j * 2048:(j + 1) * 2048], in_=o1[:])

            for j in range(3):
                rope_body(j)

        if "dbg" in io:
            nc.sync.dma_start(out=io["dbg"][:, 0:6144],
                              in_=qk_sb[:].rearrange("p a b -> p (a b)"))
            nc.sync.dma_start(out=io["dbg"][:, 6144:8192], in_=v_sb[:])

        # ---------------- phase 2: attention (qc x grp loops) ----------------
        # groups of 4 k-tiles; mask via variant table (v = qc-grp, 0=boundary)
        qk_f = qk_sb[:].rearrange("p a b -> p (a b)")        # [128, 6144]
        with tc.tile_pool(name="p3s", bufs=1) as p3s, \
             tc.tile_pool(name="pp3", bufs=1, space="PSUM") as pp3:
            kst = p3s.tile([128, 512], BF16, tag="kst")
            vst = p3s.tile([128, 512], BF16, tag="vst")
            probs = p3s.tile([128, 4, 512], BF16, tag="probs")
            red = p3s.tile([128, 512], BF16, tag="red")
            ps_sc = pp3.tile([128, 4, 512], F32, tag="ps_sc")
            ps_pv0 = pp3.tile([128, 512], F32, tag="ps_pv0")
            ps_pv1 = pp3.tile([128, 512], F32, tag="ps_pv1")
            ps_den0 = pp3.tile([1, 512], F32, tag="ps_den0")
            ps_den1 = pp3.tile([1, 512], F32, tag="ps_den1")
            pvs = (ps_pv0, ps_pv1)
            dens = (ps_den0, ps_den1)

            def grp_body(qc, grp, unroll=1):
                nc.vector.tensor_copy(kst[:], qk_f[:, ds(4096 + grp * 512, 512)])
                nc.vector.tensor_copy(vst[:], v_sb[:, ds(grp * 512, 512)])
                for h in range(2):
                    for i in range(4):
                        mm(ps_sc[:, i, :], kst[:, i * 128:(i + 1) * 128],
                           qk_f[:, ds(h * 2048 + qc * 512, 512)],
                           start=True, stop=True)
                    nc.vector.tensor_add(
                        ps_sc[:].rearrange("p a b -> p (a b)"),
                        ps_sc[:].rearrange("p a b -> p (a b)"),
                        mask_sb[:, ds((qc - grp + 3) * 2048, 2048)])
                    nc.scalar.activation(probs[:], ps_sc[:],
                                         mybir.ActivationFunctionType.Exp,
                                         scale=SCALE)
                    for i in range(4):
                        mm(pvs[h][:], vst[:, i * 128:(i + 1) * 128],
                           probs[:, i, :], start=False, stop=False)
                    with nc.allow_low_precision(
                            reason="4-term probs sum; denominators tolerate bf16"):
                        nc.vector.tensor_reduce(
                            red[:], probs[:].rearrange("p a b -> p b a"),
                            axis=mybir.AxisListType.X, op=mybir.AluOpType.add)
                    mm(dens[h][:], ones_sb[:], red[:], start=False, stop=False)

            def qc_body(qc, unroll=1):
                for h in range(2):
                    nc.vector.memset(pvs[h][:], 0.0)
                    nc.vector.memset(dens[h][:], 0.0)
                with tc.For_i(0, qc + 1, 1) as grp:
                    grp_body(qc, grp)
                for h in range(2):
                    nc.vector.tensor_copy(
                        attnT_sb[:, ds(h * 2048 + qc * 512, 512)], pvs[h][:])
                    nc.vector.reciprocal(
                        recip_sb[:, ds(h * 2048 + qc * 512, 512)], dens[h][:])

            with tc.For_i(0, 4, 1) as qc:
                qc_body(qc)

        # normalize: DRAM-bounce broadcast of 1/den, one big multiply
        rb = dram.tile([1, 4096], F32, name="rb")
        nc.sync.dma_start(out=rb[:], in_=recip_sb[:])
        with tc.tile_pool(name="bcp", bufs=1) as bcp:
            bc = bcp.tile([128, 4096], F32, tag="bc")
            nc.sync.dma_start(out=bc[:], in_=rb.to_broadcast((128, 4096)))
            nc.vector.tensor_mul(attnT_sb[:], attnT_sb[:], bc[:])

        # ---------------- phase 3: output projection (outT = wo^T attnT) -----
        # outT[n, q] = sum_f wo[f, n] attnT[f, q]; loop over (nh, qc)
        woaccT = dram.tile([128, 16, 2048], BF16, name="woaccT")
        with tc.tile_pool(name="p4s", bufs=1) as p4s, \
             tc.tile_pool(name="pp4", bufs=1, space="PSUM") as pp4:
            wst4 = p4s.tile([128, 2, 1024], BF16, tag="wst4")
            o_sb = p4s.tile([128, 8, 512], BF16, tag="osb")
            ps_wo = pp4.tile([128, 8, 512], F32, tag="ps_wo")

            def p4_body(qc, unroll=1):
                for nh in range(2):
                    # wo packed as [p, nh, f, n1024]
                    nc.vector.tensor_copy(
                        wst4[:].rearrange("p a b -> p (a b)"),
                        wo_sb[:, nh * 2048:(nh + 1) * 2048])
                    for nt in range(8):
                        for f in range(2):
                            mm(ps_wo[:, nt, :],
                               wst4[:, f, nt * 128:(nt + 1) * 128],
                               attnT_sb[:, ds(f * 2048 + qc * 512, 512)],
                               start=f == 0, stop=f == 1)
                    nc.vector.tensor_copy(o_sb[:], ps_wo[:])
                    nc.sync.dma_start(
                        out=woaccT[:, nh * 8:(nh + 1) * 8, ds(qc * 512, 512)],
                        in_=o_sb[:])

            with tc.For_i(0, 4, 1) as qc:
                p4_body(qc)

        # ---------------- ReduceScatter over cores (bf16) --------------------
        rs_out = dram.tile([16, 16, 2048], BF16, name="rsout")
        nc.gpsimd.collective_compute(
            "ReduceScatter", mybir.AluOpType.add,
            replica_groups=[list(range(NCORES))],
            ins=[woaccT.opt()], outs=[rs_out.opt()])
        nc.sync.dma_start(out=io["out"][:], in_=rs_out[:])


def build(repeat=1, num_devices=NCORES):
    key = (repeat, num_devices)
    if key in _BUILD_CACHE:
        return _BUILD_CACHE[key]
    nc = bacc.Bacc("TRN2", target_bir_lowering=False, debug=False,
                   num_devices=num_devices)
    io = {
        "xT": nc.dram_tensor("xT", [D, S], BF16, kind="ExternalInput").ap(),
        "wqkv": nc.dram_tensor("wqkv", [128, 8192], BF16,
                               kind="ExternalInput").ap(),
        "wo2": nc.dram_tensor("wo2", [128, 4096], BF16,
                              kind="ExternalInput").ap(),
        "cossin": nc.dram_tensor("cossin", [64, 4096], BF16,
                                 kind="ExternalInput").ap(),
        "maskF": nc.dram_tensor("maskF", [128, 16384], BF16,
                                kind="ExternalInput").ap(),
        "ident": nc.dram_tensor("ident", [128, 128], BF16,
                                kind="ExternalInput").ap(),
        "out": nc.dram_tensor("out", [16, 16, 2048], BF16,
                              kind="ExternalOutput").ap(),
        "dbg": nc.dram_tensor("dbg", [128, 8192], BF16,
                              kind="ExternalOutput").ap(),
    }
    with tile.TileContext(nc) as tc:
        for _ in range(repeat):
            _emit_body(nc, tc, io)
    nc.compile()
    _BUILD_CACHE[key] = nc
    return nc


def prepare_in_maps(x, wq, wk, wv, wo, freqs_cos, freqs_sin):
    bf = lambda a: np.ascontiguousarray(a).astype(NPBF16)
    x2d = np.asarray(x, dtype=np.float32).reshape(S, D)
    xT = bf(x2d.T)
    cosT = np.asarray(freqs_cos, np.float32).T                # [64, S]
    sinT = np.asarray(freqs_sin, np.float32).T
    cossin = bf(np.concatenate([cosT, sinT], axis=1))         # [64, 4096]

    perm = np.concatenate([np.arange(0, HD, 2), np.arange(1, HD, 2)])
    wq = np.asarray(wq, np.float32)
    wk = np.asarray(wk, np.float32)
    wv = np.asarray(wv, np.float32)
    wo = np.asarray(wo, np.float32)
    wqP = wq.reshape(D, H, HD)[:, :, perm]                    # [D, 16, 128]
    wkP = wk.reshape(D, KVH, HD)[:, :, perm]                  # [D, 4, 128]
    wv4 = wv.reshape(D, KVH, HD)

    # mask variant table [kl, v, i, ql]: v=0 -> boundary-quad mask, v>0 -> 0
    kl = np.arange(128)[:, None, None]
    iv = np.arange(4)[None, :, None]
    qlv = np.arange(512)[None, None, :]
    maskB = np.where(128 * iv + kl <= qlv, 0.0, NEG).reshape(128, 2048)
    maskF = np.zeros((128, 8, 2048), np.float32)
    maskF[:, 0:3, :] = NEG          # grp > qc (non-causal): mask everything
    maskF[:, 3, :] = maskB          # grp == qc: boundary quad
    maskF = bf(maskF.reshape(128, 16384))

    ident = bf(np.eye(128, dtype=np.float32))

    in_maps = []
    for c in range(NCORES):
        g = c // 2
        # wqkv packed [p, dt*512 + (q0|q1|k|v)*128 + col]
        wqkv = np.stack([wqP[:, 2 * c, :], wqP[:, 2 * c + 1, :],
                         wkP[:, g, :], wv4[:, g, :]], axis=1)  # [D, 4, 128]
        wqkv = bf(wqkv.reshape(16, 128, 512).transpose(1, 0, 2)
                  .reshape(128, 8192))
        # wo rows for this core's heads, packed [p, nh, f, n1024]
        woc = wo[256 * c:256 * c + 256, :]                    # [256, 2048]
        wo2 = bf(woc.reshape(2, 128, 2, 1024).transpose(1, 2, 0, 3)
                 .reshape(128, 4096))
        in_maps.append({
            "xT": xT,
            "wqkv": wqkv,
            "wo2": wo2,
            "cossin": cossin,
            "maskF": maskF,
            "ident": ident,
        })
    return in_maps


def assemble_output(results):
    # outT[nt*128 + 16*c + pp, q] = results[c]["out"][pp, nt, q]
    outT = np.empty((2048, 2048), np.float32)
    for c in range(NCORES):
        o = np.asarray(results[c]["out"], np.float32)         # [16, 16, 2048]
        for nt in range(16):
            outT[nt * 128 + 16 * c: nt * 128 + 16 * c + 16, :] = o[:, nt, :]
    return np.ascontiguousarray(outT.T).reshape(1, S, D)


def kernel(x, wq, wk, wv, wo, freqs_cos, freqs_sin, mask):
    nc = build()
    in_maps = prepare_in_maps(x, wq, wk, wv, wo, freqs_cos, freqs_sin)
    res = run_bass_kernel_spmd(nc, in_maps, core_ids=list(range(NCORES)))
    return assemble_output(res.results).astype(np.float32)
